# revision 12
# baseline (speedup 1.0000x reference)
"""GCN layer kernel for 8 Trainium2 NeuronCores.

out = segment_sum(edge_vals * (features @ W)[edge_src], edge_dst) + bias
    = segment_sum(edge_vals * features[edge_src], edge_dst) @ W + bias
      (W is shared across nodes, so aggregation commutes with the matmul)

Strategy (graph/data parallel per sharding hint):
- Destination nodes are sharded across 8 cores (12500 per core), in 98
  windows of 128 dsts (window = psum accumulation target).
- Edges of a window are packed densely into columns of 128 (partition =
  edge lane, NOT dst), sorted by source. Batched SWDGE dma_gather
  fetches up to 8 columns per instruction with int16 indices relative
  to a per-instruction dynamic base (base = chunk_max_src - 32767, so
  the last index of every core's chunk is non-negative -- the Q7 ucode
  treats trailing negatives as end-of-list; mid-list negatives are
  valid signed offsets). No bank structure, so no bank fragmentation.
  The full index stream is preloaded into SBUF (no per-instruction idx
  DMAs). idx=0/val=0 pads.
- Per column, one DVE tensor_scalar builds Sel[e,d] = val_e * (d == dst_e)
  from a constant iota row (is_equal + mult), and one PE matmul
  accumulates agg^T[f,d] += sum_e G[e,f] * Sel[e,d] into the window's
  PSUM bank -- scale, scatter and reduce in one op.
- Per window: copy agg^T to SBUF, PE matmul with W, DVE bias add,
  DMA out.
"""
import os
import sys
from contextlib import ExitStack

import numpy as np

_REPO = "/opt/trn_rl_repo"
if _REPO not in sys.path:
    sys.path.insert(0, _REPO)

N_NODES = 100000
N_EDGES = 3200000
DIM = 128
N_CORES = 8
P = 128
SHARD = N_NODES // N_CORES  # 12500
N_WIN = (SHARD + P - 1) // P  # 98 windows/core
SHARD_PAD = N_WIN * P  # 12544
CHUNK = int(os.environ.get("GCN_CHUNK", "8"))  # columns per dma_gather inst


def _host_schedule(edge_src, edge_dst, edge_vals):
    """Pack each core's edges into dense 128-lane columns, window-major.

    Per window: edges sorted by src, positions 0..ncols*128 (cross-core
    max, padded with idx=0/val=0). Instructions cover <=CHUNK columns;
    each gets base = max(0, global_chunk_max_src - 32767) so every
    int16 index fits and each core's last chunk index is >= 0.
    """
    core = edge_dst // SHARD
    per_core = []
    for c in range(N_CORES):
        m = core == c
        src_c = np.asarray(edge_src[m], dtype=np.int64)
        dst_c = np.asarray(edge_dst[m] - c * SHARD, dtype=np.int64)
        val_c = np.asarray(edge_vals[m], dtype=np.float32)
        w = dst_c // P
        order = np.lexsort((src_c, w))
        per_core.append(
            {"src": src_c[order], "dst": dst_c[order], "val": val_c[order],
             "w": w[order]}
        )
    counts = np.zeros((N_CORES, N_WIN), dtype=np.int64)
    for c in range(N_CORES):
        np.add.at(counts[c], per_core[c]["w"], 1)
    maxreal = np.max(counts, axis=0)
    ncols_win = np.maximum((maxreal + P - 1) // P, 1)

    cols = []  # window id per column
    win_c0 = {}
    for w in range(N_WIN):
        win_c0[w] = len(cols)
        cols += [w] * int(ncols_win[w])
    total_cols = len(cols)

    # per-core position-major cell arrays
    idx_src = np.zeros((N_CORES, total_cols * P), dtype=np.int64)  # absolute src
    has_edge = np.zeros((N_CORES, total_cols * P), dtype=bool)
    dstid = np.zeros((N_CORES, P, total_cols), dtype=np.float32)
    val = np.zeros((N_CORES, P, total_cols), dtype=np.float32)
    for c in range(N_CORES):
        pc = per_core[c]
        starts = np.concatenate([[0], np.cumsum(counts[c])])
        for w in range(N_WIN):
            lo, hi = starts[w], starts[w + 1]
            n = hi - lo
            p0 = win_c0[w] * P
            idx_src[c, p0:p0 + n] = pc["src"][lo:hi]
            has_edge[c, p0:p0 + n] = True
            ncol = int(ncols_win[w])
            pad = ncol * P - n
            d_flat = np.concatenate([pc["dst"][lo:hi] - w * P,
                                     np.zeros(pad, np.int64)]).astype(np.float32)
            v_flat = np.concatenate([pc["val"][lo:hi], np.zeros(pad, np.float32)])
            dstid[c, :, win_c0[w]:win_c0[w] + ncol] = d_flat.reshape(ncol, P).T
            val[c, :, win_c0[w]:win_c0[w] + ncol] = v_flat.reshape(ncol, P).T

    # instructions: <=CHUNK columns, never spanning a window boundary
    insts = []  # (base, col_start, n_cols)
    for w in range(N_WIN):
        nr = int(ncols_win[w])
        i = 0
        while i < nr:
            nc_i = min(CHUNK, nr - i)
            c0 = win_c0[w] + i
            p0, p1 = c0 * P, (c0 + nc_i) * P
            sl = idx_src[:, p0:p1]
            he = has_edge[:, p0:p1]
            gmax = int(sl[he].max()) if he.any() else 0
            base = max(0, gmax - 32767)
            # validity: all real idx in signed range, per-core last pos >= 0
            rel = sl - base
            assert rel[he].max() <= 32767 and rel[he].min() >= -32768, (w, i)
            for c in range(N_CORES):
                row_he = he[c]
                last = np.nonzero(row_he)[0]
                # last position of the chunk: pad (idx 0) or real edge
                if row_he[-1]:
                    assert rel[c, -1] >= 0, (w, i, c)
            insts.append((base, c0, nc_i))
            i += nc_i

    # int16 relative index streams, wrapped per instruction
    per_out = []
    for c in range(N_CORES):
        rel_all = np.zeros(total_cols * P, dtype=np.int16)
        for (base, c0, nc_i) in insts:
            p0, p1 = c0 * P, (c0 + nc_i) * P
            r = idx_src[c, p0:p1] - base
            r[~has_edge[c, p0:p1]] = 0  # pads gather row `base`, val=0
            rel_all[p0:p1] = r.astype(np.int16)
        idx16 = np.zeros((128, total_cols * 8), dtype=np.int16)
        for (base, c0, nc_i) in insts:
            L = rel_all[c0 * P:(c0 + nc_i) * P]
            wrapped = np.zeros((16, nc_i * 8), dtype=np.int16)
            ii = np.arange(nc_i * P)
            wrapped[ii % 16, ii // 16] = L
            idx16[:, c0 * 8:(c0 + nc_i) * 8] = np.tile(wrapped, (8, 1))
        per_out.append({"idx16": idx16, "dstid": dstid[c], "val": val[c]})
    return cols, insts, total_cols, per_out


def _build_nc(cols, insts, total_cols):
    import concourse.bass as bass
    import concourse.tile as tile
    from concourse import bacc, mybir, library_config

    nc = bacc.Bacc(
        "TRN2", target_bir_lowering=False, debug=False, num_devices=N_CORES,
    )
    feat_t = nc.dram_tensor("features", [N_NODES, DIM], mybir.dt.float32, kind="ExternalInput")
    idx_t = nc.dram_tensor("idx16", [P, total_cols * 8], mybir.dt.int16, kind="ExternalInput")
    dst_t = nc.dram_tensor("dstid", [P, total_cols], mybir.dt.float32, kind="ExternalInput")
    val_t = nc.dram_tensor("val", [P, total_cols], mybir.dt.float32, kind="ExternalInput")
    w_t = nc.dram_tensor("weight", [DIM, DIM], mybir.dt.float32, kind="ExternalInput")
    bias_t = nc.dram_tensor("bias_tile", [P, DIM], mybir.dt.float32, kind="ExternalInput")
    iota_t = nc.dram_tensor("iota", [P, P], mybir.dt.float32, kind="ExternalInput")
    out_t = nc.dram_tensor("outp", [N_WIN, P, DIM], mybir.dt.float32, kind="ExternalOutput")

    first_col = {}
    last_col = {}
    for ci, w in enumerate(cols):
        if w not in first_col:
            first_col[w] = ci
        last_col[w] = ci

    with tile.TileContext(nc) as tc:
        with ExitStack() as ctx:
            nc.gpsimd.load_library(library_config.mlp)
            const = ctx.enter_context(tc.tile_pool(name="const", bufs=1))
            gpool = ctx.enter_context(tc.tile_pool(name="gather", bufs=6))
            spool = ctx.enter_context(tc.tile_pool(name="sel", bufs=12))
            opool = ctx.enter_context(tc.tile_pool(name="outw", bufs=3))
            tppool = ctx.enter_context(tc.tile_pool(name="aggt", bufs=3))
            pspool = ctx.enter_context(tc.tile_pool(name="psum", bufs=2, space="PSUM"))
            psopool = ctx.enter_context(tc.tile_pool(name="pso", bufs=2, space="PSUM"))

            idx_all = const.tile([P, total_cols * 8], mybir.dt.int16)
            nc.sync.dma_start(idx_all[:], idx_t[:])
            dst_all = const.tile([P, total_cols], mybir.dt.float32)
            nc.sync.dma_start(dst_all[:], dst_t[:])
            val_all = const.tile([P, total_cols], mybir.dt.float32)
            nc.sync.dma_start(val_all[:], val_t[:])
            w_tile = const.tile([DIM, DIM], mybir.dt.float32)
            nc.sync.dma_start(w_tile[:], w_t[:])
            bias_tile = const.tile([P, DIM], mybir.dt.float32)
            nc.sync.dma_start(bias_tile[:], bias_t[:])
            iota = const.tile([P, P], mybir.dt.float32)
            nc.sync.dma_start(iota[:], iota_t[:])

            ps_t = None
            for (base, c0, nc_i) in insts:
                G = gpool.tile([P, nc_i * P], mybir.dt.float32, name="G")
                g3 = G[:].rearrange("p (c f) -> p c f", f=P)
                nc.gpsimd.dma_gather(
                    out_ap=g3,
                    in_ap=feat_t[base:][:],
                    idxs_ap=idx_all[:, c0 * 8:(c0 + nc_i) * 8],
                    num_idxs=nc_i * P,
                    num_idxs_reg=nc_i * P,
                    elem_size=DIM,
                )
                for j in range(nc_i):
                    ci = c0 + j
                    w = cols[ci]
                    sel = spool.tile([P, P], mybir.dt.float32, name="sel")
                    nc.vector.tensor_scalar(
                        out=sel[:], in0=iota[:],
                        scalar1=dst_all[:, ci:ci + 1],
                        scalar2=val_all[:, ci:ci + 1],
                        op0=mybir.AluOpType.is_equal,
                        op1=mybir.AluOpType.mult,
                    )
                    if ci == first_col[w]:
                        ps_t = pspool.tile([P, P], mybir.dt.float32, name="pst")
                    # agg^T[f, d] += sum_e G[e, f] * Sel[e, d]
                    nc.tensor.matmul(
                        out=ps_t[:],
                        lhsT=G[:, j * P:(j + 1) * P],
                        rhs=sel[:],
                        start=(ci == first_col[w]),
                        stop=(ci == last_col[w]),
                    )
                    if ci == last_col[w]:
                        agg_tr = tppool.tile([P, P], mybir.dt.float32)
                        nc.vector.tensor_copy(agg_tr[:], ps_t[:])
                        ps_o = psopool.tile([P, DIM], mybir.dt.float32, name="pso")
                        nc.tensor.matmul(
                            out=ps_o[:], lhsT=agg_tr[:], rhs=w_tile[:],
                            start=True, stop=True,
                        )
                        ow = opool.tile([P, DIM], mybir.dt.float32)
                        nc.vector.tensor_add(ow[:], ps_o[:], bias_tile[:])
                        nc.sync.dma_start(out_t[w], ow[:])
    nc.compile()
    return nc


def kernel(features, edge_src, edge_dst, edge_vals, weight, bias):
    features = np.ascontiguousarray(np.asarray(features), dtype=np.float32)
    edge_src = np.asarray(edge_src).astype(np.int64)
    edge_dst = np.asarray(edge_dst).astype(np.int64)
    edge_vals = np.asarray(edge_vals).astype(np.float32)
    weight = np.asarray(weight).astype(np.float32)
    bias = np.asarray(bias).astype(np.float32)

    cols, insts, total_cols, per_core = _host_schedule(edge_src, edge_dst, edge_vals)
    nc = _build_nc(cols, insts, total_cols)

    from concourse.bass_utils import run_bass_kernel_spmd

    bias_tile = np.tile(bias[None, :], (P, 1)).astype(np.float32)
    iota = np.tile(np.arange(P, dtype=np.float32)[None, :], (P, 1))
    in_maps = []
    for c in range(N_CORES):
        in_maps.append(
            {
                "features": features,
                "idx16": per_core[c]["idx16"],
                "dstid": per_core[c]["dstid"],
                "val": per_core[c]["val"],
                "weight": weight,
                "bias_tile": bias_tile,
                "iota": iota,
            }
        )
    trace = os.environ.get("GCN_TRACE", "0") == "1"
    res = None
    for attempt in range(3):
        try:
            res = run_bass_kernel_spmd(
                nc, in_maps, core_ids=list(range(N_CORES)), trace=trace
            )
            break
        except Exception:
            if attempt == 2:
                raise
            import time as _time

            _time.sleep(15.0)  # transient device flakes recover across retries
    if trace:
        print(f"HW exec time: {res.exec_time_ns} ns")
        kernel.last_exec_time_ns = res.exec_time_ns

    out = np.empty((N_NODES, DIM), dtype=np.float32)
    for c in range(N_CORES):
        op = res.results[c]["outp"].reshape(SHARD_PAD, DIM)
        out[c * SHARD:(c + 1) * SHARD] = op[:SHARD]
    return out


kernel.last_exec_time_ns = None


# revision 13
# speedup vs baseline: 1.0047x; 1.0047x over previous
"""GCN layer kernel for 8 Trainium2 NeuronCores.

out = segment_sum(edge_vals * (features @ W)[edge_src], edge_dst) + bias
    = segment_sum(edge_vals * features[edge_src], edge_dst) @ W + bias
      (W is shared across nodes, so aggregation commutes with the matmul)

Strategy (graph/data parallel per sharding hint):
- Destination nodes are sharded across 8 cores (12500 per core), in 98
  windows of 128 dsts (window = psum accumulation target).
- Edges of a window are packed densely into columns of 128 (partition =
  edge lane, NOT dst), sorted by source. Batched SWDGE dma_gather
  fetches up to 8 columns per instruction with int16 indices relative
  to a per-instruction dynamic base (base = chunk_max_src - 32767, so
  the last index of every core's chunk is non-negative -- the Q7 ucode
  treats trailing negatives as end-of-list; mid-list negatives are
  valid signed offsets). No bank structure, so no bank fragmentation.
  Index tiles are streamed per instruction. idx=0/val=0 pads.
- Per column, one DVE tensor_scalar builds Sel[e,d] = val_e * (d == dst_e)
  from a constant iota row (is_equal + mult), and one PE matmul
  accumulates agg^T[f,d] += sum_e G[e,f] * Sel[e,d] into the window's
  PSUM bank -- scale, scatter and reduce in one op.
- Per window: copy agg^T to SBUF, PE matmul with W, DVE bias add,
  DMA out.
"""
import os
import sys
from contextlib import ExitStack

import numpy as np

_REPO = "/opt/trn_rl_repo"
if _REPO not in sys.path:
    sys.path.insert(0, _REPO)

N_NODES = 100000
N_EDGES = 3200000
DIM = 128
N_CORES = 8
P = 128
SHARD = N_NODES // N_CORES  # 12500
N_WIN = (SHARD + P - 1) // P  # 98 windows/core
SHARD_PAD = N_WIN * P  # 12544
CHUNK = int(os.environ.get("GCN_CHUNK", "8"))  # columns per dma_gather inst


def _host_schedule(edge_src, edge_dst, edge_vals):
    """Pack each core's edges into dense 128-lane columns, window-major.

    Per window: edges sorted by src, positions 0..ncols*128 (cross-core
    max, padded with idx=0/val=0). Instructions cover <=CHUNK columns;
    each gets base = max(0, global_chunk_max_src - 32767) so every
    int16 index fits and each core's last chunk index is >= 0.
    """
    core = edge_dst // SHARD
    per_core = []
    for c in range(N_CORES):
        m = core == c
        src_c = np.asarray(edge_src[m], dtype=np.int64)
        dst_c = np.asarray(edge_dst[m] - c * SHARD, dtype=np.int64)
        val_c = np.asarray(edge_vals[m], dtype=np.float32)
        w = dst_c // P
        order = np.lexsort((src_c, w))
        per_core.append(
            {"src": src_c[order], "dst": dst_c[order], "val": val_c[order],
             "w": w[order]}
        )
    counts = np.zeros((N_CORES, N_WIN), dtype=np.int64)
    for c in range(N_CORES):
        np.add.at(counts[c], per_core[c]["w"], 1)
    maxreal = np.max(counts, axis=0)
    ncols_win = np.maximum((maxreal + P - 1) // P, 1)

    cols = []  # window id per column
    win_c0 = {}
    for w in range(N_WIN):
        win_c0[w] = len(cols)
        cols += [w] * int(ncols_win[w])
    total_cols = len(cols)

    # per-core position-major cell arrays
    idx_src = np.zeros((N_CORES, total_cols * P), dtype=np.int64)  # absolute src
    has_edge = np.zeros((N_CORES, total_cols * P), dtype=bool)
    dstid = np.zeros((N_CORES, P, total_cols), dtype=np.float32)
    val = np.zeros((N_CORES, P, total_cols), dtype=np.float32)
    for c in range(N_CORES):
        pc = per_core[c]
        starts = np.concatenate([[0], np.cumsum(counts[c])])
        for w in range(N_WIN):
            lo, hi = starts[w], starts[w + 1]
            n = hi - lo
            p0 = win_c0[w] * P
            idx_src[c, p0:p0 + n] = pc["src"][lo:hi]
            has_edge[c, p0:p0 + n] = True
            ncol = int(ncols_win[w])
            pad = ncol * P - n
            d_flat = np.concatenate([pc["dst"][lo:hi] - w * P,
                                     np.zeros(pad, np.int64)]).astype(np.float32)
            v_flat = np.concatenate([pc["val"][lo:hi], np.zeros(pad, np.float32)])
            dstid[c, :, win_c0[w]:win_c0[w] + ncol] = d_flat.reshape(ncol, P).T
            val[c, :, win_c0[w]:win_c0[w] + ncol] = v_flat.reshape(ncol, P).T

    # instructions: <=CHUNK columns, never spanning a window boundary
    insts = []  # (base, col_start, n_cols)
    for w in range(N_WIN):
        nr = int(ncols_win[w])
        i = 0
        while i < nr:
            nc_i = min(CHUNK, nr - i)
            c0 = win_c0[w] + i
            p0, p1 = c0 * P, (c0 + nc_i) * P
            sl = idx_src[:, p0:p1]
            he = has_edge[:, p0:p1]
            gmax = int(sl[he].max()) if he.any() else 0
            base = max(0, gmax - 32767)
            # validity: all real idx in signed range, per-core last pos >= 0
            rel = sl - base
            assert rel[he].max() <= 32767 and rel[he].min() >= -32768, (w, i)
            for c in range(N_CORES):
                row_he = he[c]
                last = np.nonzero(row_he)[0]
                # last position of the chunk: pad (idx 0) or real edge
                if row_he[-1]:
                    assert rel[c, -1] >= 0, (w, i, c)
            insts.append((base, c0, nc_i))
            i += nc_i

    # int16 relative index streams, wrapped per instruction
    per_out = []
    for c in range(N_CORES):
        rel_all = np.zeros(total_cols * P, dtype=np.int16)
        for (base, c0, nc_i) in insts:
            p0, p1 = c0 * P, (c0 + nc_i) * P
            r = idx_src[c, p0:p1] - base
            r[~has_edge[c, p0:p1]] = 0  # pads gather row `base`, val=0
            rel_all[p0:p1] = r.astype(np.int16)
        idx16 = np.zeros((128, total_cols * 8), dtype=np.int16)
        for (base, c0, nc_i) in insts:
            L = rel_all[c0 * P:(c0 + nc_i) * P]
            wrapped = np.zeros((16, nc_i * 8), dtype=np.int16)
            ii = np.arange(nc_i * P)
            wrapped[ii % 16, ii // 16] = L
            idx16[:, c0 * 8:(c0 + nc_i) * 8] = np.tile(wrapped, (8, 1))
        per_out.append({"idx16": idx16, "dstid": dstid[c], "val": val[c]})
    return cols, insts, total_cols, per_out


def _build_nc(cols, insts, total_cols):
    import concourse.bass as bass
    import concourse.tile as tile
    from concourse import bacc, mybir, library_config

    nc = bacc.Bacc(
        "TRN2", target_bir_lowering=False, debug=False, num_devices=N_CORES,
    )
    feat_t = nc.dram_tensor("features", [N_NODES, DIM], mybir.dt.float32, kind="ExternalInput")
    idx_t = nc.dram_tensor("idx16", [P, total_cols * 8], mybir.dt.int16, kind="ExternalInput")
    dst_t = nc.dram_tensor("dstid", [P, total_cols], mybir.dt.float32, kind="ExternalInput")
    val_t = nc.dram_tensor("val", [P, total_cols], mybir.dt.float32, kind="ExternalInput")
    w_t = nc.dram_tensor("weight", [DIM, DIM], mybir.dt.float32, kind="ExternalInput")
    bias_t = nc.dram_tensor("bias_tile", [P, DIM], mybir.dt.float32, kind="ExternalInput")
    iota_t = nc.dram_tensor("iota", [P, P], mybir.dt.float32, kind="ExternalInput")
    out_t = nc.dram_tensor("outp", [N_WIN, P, DIM], mybir.dt.float32, kind="ExternalOutput")

    first_col = {}
    last_col = {}
    for ci, w in enumerate(cols):
        if w not in first_col:
            first_col[w] = ci
        last_col[w] = ci

    with tile.TileContext(nc) as tc:
        with ExitStack() as ctx:
            nc.gpsimd.load_library(library_config.mlp)
            const = ctx.enter_context(tc.tile_pool(name="const", bufs=1))
            ipool = ctx.enter_context(tc.tile_pool(name="idx", bufs=8))
            gpool = ctx.enter_context(tc.tile_pool(name="gather", bufs=6))
            spool = ctx.enter_context(tc.tile_pool(name="sel", bufs=12))
            opool = ctx.enter_context(tc.tile_pool(name="outw", bufs=3))
            tppool = ctx.enter_context(tc.tile_pool(name="aggt", bufs=3))
            pspool = ctx.enter_context(tc.tile_pool(name="psum", bufs=2, space="PSUM"))
            psopool = ctx.enter_context(tc.tile_pool(name="pso", bufs=2, space="PSUM"))

            dst_all = const.tile([P, total_cols], mybir.dt.float32)
            nc.sync.dma_start(dst_all[:], dst_t[:])
            val_all = const.tile([P, total_cols], mybir.dt.float32)
            nc.sync.dma_start(val_all[:], val_t[:])
            w_tile = const.tile([DIM, DIM], mybir.dt.float32)
            nc.sync.dma_start(w_tile[:], w_t[:])
            bias_tile = const.tile([P, DIM], mybir.dt.float32)
            nc.sync.dma_start(bias_tile[:], bias_t[:])
            iota = const.tile([P, P], mybir.dt.float32)
            nc.sync.dma_start(iota[:], iota_t[:])

            ps_t = None
            for (base, c0, nc_i) in insts:
                idxs = ipool.tile([P, nc_i * 8], mybir.dt.int16, name="idxs")
                nc.sync.dma_start(idxs[:], idx_t[:, c0 * 8:(c0 + nc_i) * 8])
                G = gpool.tile([P, nc_i * P], mybir.dt.float32, name="G")
                g3 = G[:].rearrange("p (c f) -> p c f", f=P)
                nc.gpsimd.dma_gather(
                    out_ap=g3,
                    in_ap=feat_t[base:][:],
                    idxs_ap=idxs[:],
                    num_idxs=nc_i * P,
                    num_idxs_reg=nc_i * P,
                    elem_size=DIM,
                )
                for j in range(nc_i):
                    ci = c0 + j
                    w = cols[ci]
                    sel = spool.tile([P, P], mybir.dt.float32, name="sel")
                    nc.vector.tensor_scalar(
                        out=sel[:], in0=iota[:],
                        scalar1=dst_all[:, ci:ci + 1],
                        scalar2=val_all[:, ci:ci + 1],
                        op0=mybir.AluOpType.is_equal,
                        op1=mybir.AluOpType.mult,
                    )
                    if ci == first_col[w]:
                        ps_t = pspool.tile([P, P], mybir.dt.float32, name="pst")
                    # agg^T[f, d] += sum_e G[e, f] * Sel[e, d]
                    nc.tensor.matmul(
                        out=ps_t[:],
                        lhsT=G[:, j * P:(j + 1) * P],
                        rhs=sel[:],
                        start=(ci == first_col[w]),
                        stop=(ci == last_col[w]),
                    )
                    if ci == last_col[w]:
                        agg_tr = tppool.tile([P, P], mybir.dt.float32)
                        nc.vector.tensor_copy(agg_tr[:], ps_t[:])
                        ps_o = psopool.tile([P, DIM], mybir.dt.float32, name="pso")
                        nc.tensor.matmul(
                            out=ps_o[:], lhsT=agg_tr[:], rhs=w_tile[:],
                            start=True, stop=True,
                        )
                        ow = opool.tile([P, DIM], mybir.dt.float32)
                        nc.vector.tensor_add(ow[:], ps_o[:], bias_tile[:])
                        nc.sync.dma_start(out_t[w], ow[:])
    nc.compile()
    return nc


def kernel(features, edge_src, edge_dst, edge_vals, weight, bias):
    features = np.ascontiguousarray(np.asarray(features), dtype=np.float32)
    edge_src = np.asarray(edge_src).astype(np.int64)
    edge_dst = np.asarray(edge_dst).astype(np.int64)
    edge_vals = np.asarray(edge_vals).astype(np.float32)
    weight = np.asarray(weight).astype(np.float32)
    bias = np.asarray(bias).astype(np.float32)

    cols, insts, total_cols, per_core = _host_schedule(edge_src, edge_dst, edge_vals)
    nc = _build_nc(cols, insts, total_cols)

    from concourse.bass_utils import run_bass_kernel_spmd

    bias_tile = np.tile(bias[None, :], (P, 1)).astype(np.float32)
    iota = np.tile(np.arange(P, dtype=np.float32)[None, :], (P, 1))
    in_maps = []
    for c in range(N_CORES):
        in_maps.append(
            {
                "features": features,
                "idx16": per_core[c]["idx16"],
                "dstid": per_core[c]["dstid"],
                "val": per_core[c]["val"],
                "weight": weight,
                "bias_tile": bias_tile,
                "iota": iota,
            }
        )
    trace = os.environ.get("GCN_TRACE", "0") == "1"
    res = None
    for attempt in range(3):
        try:
            res = run_bass_kernel_spmd(
                nc, in_maps, core_ids=list(range(N_CORES)), trace=trace
            )
            break
        except Exception:
            if attempt == 2:
                raise
            import time as _time

            _time.sleep(15.0)  # transient device flakes recover across retries
    if trace:
        print(f"HW exec time: {res.exec_time_ns} ns")
        kernel.last_exec_time_ns = res.exec_time_ns

    out = np.empty((N_NODES, DIM), dtype=np.float32)
    for c in range(N_CORES):
        op = res.results[c]["outp"].reshape(SHARD_PAD, DIM)
        out[c * SHARD:(c + 1) * SHARD] = op[:SHARD]
    return out


kernel.last_exec_time_ns = None
